# revision 7
# baseline (speedup 1.0000x reference)
"""ELPH edge-aware GNN message passing on 8 Trainium2 NeuronCores.

Strategy (edge-parallel, dst-sorted, refined from the 258us baseline):
  - Sort edges by destination and shard them so core c owns all edges whose
    dst lies in its 12500-node range. The per-device scatter-add then needs
    no all-reduce: each core aggregates only into its own node slice.
  - Within a core, edges are grouped into 128-node destination windows
    ("blocks"); each 128-edge chunk's scatter-add is one PSUM-accumulated
    matmul against a 0/1 destination-selection matrix A, applied before the
    W2 projection (T1 = sum_e relu(h)_e A_e per window).
  - NEW v4: A is shipped as FP8 (0/1 exact, half the bytes of the v1 bf16
    amat) on the otherwise-idle GpSimd SWDGE queue and fed to the scatter
    matmul directly (PE upconverts both operands to FP22). On-device
    is_equal A-builds (v2) cost ~164ns/chunk of DVE time - more than the
    DVE+ACT budget allows.
  - NEW v2: the edge-feature matmul is eliminated. Host folds the 5-dim
    [log1p(ef); 1] contribution into the streamed endpoint columns via the
    exact rank-5 correction M = inv(W1ab^T) @ [W1c; b1]^T, so
    h = W1ab^T (xsd + M ef5) reproduces the full first layer in ONE matmul
    per chunk (2 matmuls/chunk total incl. scatter, down from 3).
  - NEW v2: relu drains run at FD=512 (4 chunks per op) to amortize the
    PSUM-read overhead, balanced across DVE and ACT; A-builds are split
    DVE/GpSimd to keep all engines below the PE roofline.
  - EVERY matmul keeps the full (128,128) PE tile config (all operands
    zero-padded to K=128 / M=128): mixing tile configs forces a PE array
    reconfig (~215ns/MM) and keeps the HAM clock gate cold.
"""
import numpy as np
import ml_dtypes

import concourse.bass as bass
import concourse.mybir as mybir
import concourse.tile as tile
from concourse import bacc
from concourse.bass_utils import run_bass_kernel_spmd

N_NODES = 100000
D_NODE = 64
D_EDGE = 4
H_MSG = 128
H_UPD = 128
N_CORES = 8
N_CORE = N_NODES // N_CORES          # 12500
BLK = 128
N_BLOCKS = (N_CORE + BLK - 1) // BLK  # 98
N_CORE_PAD = N_BLOCKS * BLK           # 12544
P = 128
G = 4                                 # chunks per drain group / PSUM bank
ST = 8                                # chunks per supertile (1024 edges)
OB = 4                                # blocks per batched output store
LAG = 3                               # groups between h-MM and scatter
POOL_A = False                        # GpSimd tensor ops are ~2.2us each on HW: keep A-builds on DVE

BF16 = mybir.dt.bfloat16
F32 = mybir.dt.float32
FP8 = mybir.dt.float8e4
nbf16 = ml_dtypes.bfloat16
nfp8 = ml_dtypes.float8_e4m3fn


def _install_trace_hook_if_possible():
    """Best-effort antenv.axon_hooks shim; only matters if BASS_TRACE is set."""
    import sys
    import types
    try:
        import antenv
        import antenv.axon_hooks  # noqa: F401
        return
    except Exception:
        pass
    try:
        import antenv
        from trn_agent_boot.trn_boot import _ntff_profile_via_ctypes
        mod = types.ModuleType("antenv.axon_hooks")
        mod._hook = _ntff_profile_via_ctypes("/opt/axon/libaxon_pjrt.so")
        mod.set_axon_ntff_profile_hook = lambda h: setattr(mod, "_hook", h)
        mod.get_axon_ntff_profile_hook = lambda: mod._hook
        sys.modules["antenv.axon_hooks"] = mod
        antenv.axon_hooks = mod
    except Exception:
        import os
        os.environ["BASS_NEVER_TRACE"] = "1"


def _build_program(chunk_meta, C, E_pad):
    """chunk_meta: list of (block_id, is_first_in_block, is_last_in_block)."""
    nc = bacc.Bacc("TRN2", target_bir_lowering=False, debug=False)

    xsdt = nc.declare_dram_parameter("xsdt", [P, E_pad], BF16, isOutput=False)
    amat = nc.declare_dram_parameter("amat", [P, E_pad], FP8, isOutput=False)
    xtp = nc.declare_dram_parameter("xtp", [P, N_CORE_PAD], BF16, isOutput=False)
    w1ab = nc.declare_dram_parameter("w1ab", [P, H_MSG], BF16, isOutput=False)
    vw = nc.declare_dram_parameter("vw", [H_MSG, H_UPD], BF16, isOutput=False)
    u1a = nc.declare_dram_parameter("u1a", [P, H_UPD], BF16, isOutput=False)
    u2 = nc.declare_dram_parameter("u2", [H_UPD, P], BF16, isOutput=False)
    bu1c = nc.declare_dram_parameter("bu1c", [H_UPD, 1], F32, isOutput=False)
    bu2c = nc.declare_dram_parameter("bu2c", [D_NODE, 1], F32, isOutput=False)
    outt = nc.declare_dram_parameter("outt", [D_NODE, N_CORE_PAD], F32, isOutput=True)
    warm_out = nc.declare_dram_parameter("warm_out", [P, 8], F32, isOutput=True)

    n_g = C // G
    n_blocks_prog = chunk_meta[-1][0] + 1
    with tile.TileContext(nc) as tc:
        with (
            tc.tile_pool(name="const", bufs=1) as cpool,
            tc.tile_pool(name="xsd", bufs=6) as xsd_pool,
            tc.tile_pool(name="hh", bufs=5) as h_pool,
            tc.tile_pool(name="sel", bufs=4) as a_pool,
            tc.tile_pool(name="upd", bufs=2) as upd_pool,
            tc.tile_pool(name="peh", bufs=3, space="PSUM") as peh_pool,
            tc.tile_pool(name="pt1", bufs=2, space="PSUM") as pt1_pool,
            tc.tile_pool(name="pblk", bufs=3, space="PSUM") as pblk_pool,
        ):
            def cload(shape, dt_, param):
                t = cpool.tile(shape, dt_, tag=param.name, name=param.name + "_sb")
                nc.scalar.dma_start(out=t[:], in_=param[:])
                return t

            w1ab_sb = cload([P, H_MSG], BF16, w1ab)

            xtp_sb = cpool.tile([P, N_CORE_PAD], BF16, tag="xtp",
                                name="xtp_sb")
            nc.scalar.dma_start(out=xtp_sb[:, 0:1024], in_=xtp[:, 0:1024])
            u1a_sb = cload([P, H_UPD], BF16, u1a)
            v_sb = cload([H_MSG, H_UPD], BF16, vw)
            u2_sb = cload([H_UPD, P], BF16, u2)
            bu1_sb = cload([H_UPD, 1], F32, bu1c)
            bu2_sb = cload([D_NODE, 1], F32, bu2c)
            nc.scalar.dma_start(out=xtp_sb[:, 1024:2048],
                                in_=xtp[:, 1024:2048])
            nc.scalar.dma_start(out=xtp_sb[:, 2048:N_CORE_PAD],
                                in_=xtp[:, 2048:N_CORE_PAD])

            warmo = upd_pool.tile([P, 8], F32, tag="warmo", name="warmo")
            nc.vector.memset(warmo[:], 0)
            nc.scalar.dma_start(out=warm_out[:], in_=warmo[:])

            state = {"p_t1": None}
            from collections import deque
            tailq = deque()

            def emit_agg(pc, h2, hks, am, aks):
                blk_id, first, last = chunk_meta[pc]
                par = blk_id % 2
                if first and par == 0:
                    state["p_t1"] = pt1_pool.tile([H_MSG, 2 * P], F32,
                                                  space="PSUM",
                                                  tag="p_t1", name="p_t1")
                p_t1 = state["p_t1"]
                nc.tensor.matmul(out=p_t1[:, par * P:(par + 1) * P],
                                 lhsT=h2[:, hks], rhs=am[:, aks],
                                 start=first, stop=last)
                if not (last and (par == 1 or blk_id == n_blocks_prog - 1)):
                    return

                wid = 2 * P  # superblock = 2 blocks
                sb_id = blk_id // 2

                def stage1(sb_id=sb_id, p_t1=p_t1):
                    t1_sb = h_pool.tile([H_MSG, wid], BF16, tag="t1",
                                        name="t1_sb")
                    nc.scalar.activation(
                        out=t1_sb[:], in_=p_t1[:],
                        func=mybir.ActivationFunctionType.Copy)
                    return t1_sb

                def stage2(t1_sb, sb_id=sb_id):
                    # agg never materialized: U1b^T(W2^T T1 + b2 deg) folded
                    # host-side into V = W2 @ U1b (rhs = T1) + u1a deg row
                    p_uh = pblk_pool.tile([H_UPD, wid], F32, space="PSUM",
                                          tag="pag", name="p_uh")
                    nc.tensor.matmul(
                        out=p_uh[:], lhsT=u1a_sb[:],
                        rhs=xtp_sb[:, sb_id * wid:(sb_id + 1) * wid],
                        start=True, stop=False)
                    nc.tensor.matmul(out=p_uh[:], lhsT=v_sb[:], rhs=t1_sb[:],
                                     start=False, stop=True)
                    return p_uh

                def stage3(p_uh, sb_id=sb_id):
                    ru = upd_pool.tile([H_UPD, wid], BF16, tag="ru", name="ru")
                    nc.scalar.activation(
                        out=ru[:], in_=p_uh[:],
                        func=mybir.ActivationFunctionType.Relu,
                        bias=bu1_sb[:, :1])
                    p_o = pblk_pool.tile([P, wid], F32, space="PSUM",
                                         tag="pag", name="p_o")
                    nc.tensor.matmul(out=p_o[:], lhsT=u2_sb[:], rhs=ru[:],
                                     start=True, stop=True)
                    ob_i = sb_id // 2
                    ob_k = sb_id % 2
                    if ob_k == 0:
                        state["osb"] = upd_pool.tile(
                            [D_NODE, 2 * wid], F32, tag="osb", name="osb")
                    osb = state["osb"]
                    # bu2 is added host-side after the gather
                    nc.vector.tensor_copy(
                        out=osb[:, ob_k * wid:(ob_k + 1) * wid],
                        in_=p_o[0:D_NODE, :])
                    if ob_k == 1 or sb_id == (n_blocks_prog - 1) // 2:
                        nc.sync.dma_start(
                            out=outt[:, ob_i * 2 * wid:
                                     ob_i * 2 * wid + (ob_k + 1) * wid],
                            in_=osb[:, 0:(ob_k + 1) * wid])

                tailq.append([0, stage1, stage2, stage3, [None]])

            def pump_tailq(budget=3):
                done = 0
                while tailq and done < budget:
                    ent = tailq[0]
                    if ent[0] == 0:
                        ent[0] = 1
                        return
                    if ent[1] is not None:
                        ent[4][0] = ent[1]()
                        ent[1] = None
                    elif ent[2] is not None:
                        ent[4][0] = ent[2](ent[4][0])
                        ent[2] = None
                    else:
                        ent[3](ent[4][0])
                        tailq.popleft()
                    done += 1

            groups = {}

            def emit_group_tail(g):
                # drain (relu) of the previous group's PSUM into bf16 h2
                prev = groups[g - 1] if g >= 1 and (g - 1) in groups else None
                if prev is not None and "h2" not in prev:
                    h2 = h_pool.tile([P, G * H_MSG], BF16, tag="h", name="h2")
                    if g % 2 == 0:
                        nc.vector.tensor_scalar(
                            out=h2[:], in0=prev["p2"][:], scalar1=0.0,
                            scalar2=None, op0=mybir.AluOpType.max)
                    else:
                        nc.scalar.activation(
                            out=h2[:], in_=prev["p2"][:],
                            func=mybir.ActivationFunctionType.Relu)
                    prev["h2"] = h2
                if g >= LAG and (g - LAG) in groups:
                    lag = groups.pop(g - LAG)
                    for k in range(G):
                        c = (g - LAG) * G + k
                        aks = slice(lag["off"] + k * P,
                                    lag["off"] + (k + 1) * P)
                        emit_agg(c, lag["h2"], slice(k * H_MSG, (k + 1) * H_MSG),
                                 lag["am"], aks)
                    pump_tailq(budget=3)

            for g in range(n_g):
                c0 = g * G
                if c0 % ST == 0:
                    e0 = c0 * P
                    w = ST * P
                    xsd_sb = xsd_pool.tile([P, w], BF16, tag="xsd",
                                           name="xsd_sb")
                    eng = nc.sync if (c0 // ST) % 2 == 0 else nc.scalar
                    eng.dma_start(out=xsd_sb[:], in_=xsdt[:, e0:e0 + w])
                    am_sb = a_pool.tile([P, w], FP8, tag="A", name="am_sb")
                    nc.gpsimd.dma_start(out=am_sb[:], in_=amat[:, e0:e0 + w])
                    cur = {"xsd": xsd_sb, "am": am_sb}
                st_off = (c0 % ST) * P

                p2 = peh_pool.tile([P, G * H_MSG], F32, space="PSUM",
                                   tag="p_eh", name="p2")
                for k in range(G):
                    ks = slice(st_off + k * P, st_off + (k + 1) * P)
                    hs = slice(k * H_MSG, (k + 1) * H_MSG)
                    nc.tensor.matmul(out=p2[:, hs], lhsT=cur["xsd"][:, ks],
                                     rhs=w1ab_sb[:], start=True, stop=True)
                groups[g] = {"p2": p2, "am": cur["am"], "off": st_off}
                emit_group_tail(g)

            # drain: advance all pending blocks round-robin, one stage per
            # pass, so stages of consecutive blocks overlap across engines
            for g in range(n_g, n_g + LAG + 1):
                emit_group_tail(g)
            while tailq:
                for ent in list(tailq):
                    if ent[1] is not None:
                        ent[4][0] = ent[1]()
                        ent[1] = None
                    elif ent[2] is not None:
                        ent[4][0] = ent[2](ent[4][0])
                        ent[2] = None
                    elif ent[3] is not None:
                        ent[3](ent[4][0])
                        ent[3] = None
                while tailq and tailq[0][3] is None:
                    tailq.popleft()
    if not nc.is_finalized():
        nc.finalize()
    return nc


def kernel(x, edge_index, edge_features, W1, b1, W2, b2, U1, bu1, U2, bu2):
    x = np.asarray(x, dtype=np.float32)
    ei = np.asarray(edge_index).astype(np.int64)
    ef = np.asarray(edge_features, dtype=np.float32)
    src, dst = ei[0], ei[1]
    E = src.shape[0]

    order = np.argsort(dst, kind="stable")
    src_s, dst_s, ef_s = src[order], dst[order], ef[order]

    core_of = dst_s // N_CORE
    blk_of = (dst_s % N_CORE) // BLK

    # per-(core, block) edge counts -> shared chunk schedule.
    # Each core maps its rank-k largest block to program slot k, so the
    # shared per-slot chunk count is the max over ALIGNED sorted profiles
    # (near-identical across cores) instead of the max over independent
    # Poisson draws: padding drops from ~15% to ceil-waste (~6%).
    cnt = np.zeros((N_CORES, N_BLOCKS), dtype=np.int64)
    np.add.at(cnt, (core_of, blk_of), 1)
    nbc = np.maximum(1, (cnt + P - 1) // P)          # [core, block] chunks
    blk_order = np.argsort(-nbc, axis=1, kind="stable")  # core's slot->block
    sorted_nb = np.take_along_axis(nbc, blk_order, axis=1)
    NB = sorted_nb.max(axis=0)                       # chunks per SLOT
    pad4 = (-NB.sum()) % ST
    NB[-1] += pad4
    C = int(NB.sum())
    E_pad = C * P
    blk_chunk0 = np.concatenate([[0], np.cumsum(NB)[:-1]])  # per SLOT

    chunk_meta = []
    for s in range(N_BLOCKS):
        for j in range(int(NB[s])):
            chunk_meta.append((s, j == 0, j == int(NB[s]) - 1))

    # rank-5 fold of [log1p(ef); 1] @ [W1c; b1] into the xsd stream:
    # h = W1ab^T (xsd + M ef5) with M = inv(W1ab^T) [W1c; b1]^T
    W1 = np.asarray(W1)
    W1ab64 = W1[:2 * D_NODE].astype(np.float64)
    rhs5 = np.concatenate(
        [W1[2 * D_NODE:].astype(np.float64),
         np.asarray(b1, dtype=np.float64).reshape(1, H_MSG)], axis=0)
    M5 = np.linalg.solve(W1ab64.T, rhs5.T)            # [128, 5]

    xbf = x.astype(nbf16)
    w1ab_h = np.ascontiguousarray(W1[:2 * D_NODE]).astype(nbf16)
    U1 = np.asarray(U1)
    # V = W2 @ U1b: lets U1b consume T1 directly (agg never materialized)
    v_h = (np.asarray(W2, dtype=np.float64)
           @ U1[D_NODE:].astype(np.float64)).astype(np.float32)
    v_h = v_h.astype(nbf16)
    u1a_h = np.zeros((P, H_UPD), dtype=np.float32)
    u1a_h[:D_NODE] = U1[:D_NODE]
    # agg = W2^T T1 + b2 (x) deg feeds U1b; the bias part equals
    # (U1b^T b2) (x) deg, absorbed here via xtp row 64 (= deg)
    u1a_h[D_NODE] = U1[D_NODE:].T @ np.asarray(b2, dtype=np.float32)
    u1a_h = u1a_h.astype(nbf16)
    u2_h = np.zeros((H_UPD, P), dtype=np.float32)
    u2_h[:, :D_NODE] = np.asarray(U2)
    u2_h = u2_h.astype(nbf16)
    bu1_h = np.asarray(bu1, dtype=np.float32).reshape(H_UPD, 1)
    bu2_h = np.asarray(bu2, dtype=np.float32).reshape(D_NODE, 1)

    # per-core edge slot assignment (vectorized): edge -> padded slot index
    in_maps = []
    for c in range(N_CORES):
        m = core_of == c
        eb = blk_of[m]
        # edges are dst-sorted, so eb is sorted; rank within block =
        # position - first position of that block
        first_pos = np.searchsorted(eb, np.arange(N_BLOCKS), side="left")
        rank = np.arange(eb.shape[0]) - first_pos[eb]
        slot_of_blk = np.empty(N_BLOCKS, dtype=np.int64)
        slot_of_blk[blk_order[c]] = np.arange(N_BLOCKS)
        slot = (blk_chunk0[slot_of_blk[eb]] * P + rank).astype(np.int64)

        e_src = src_s[m]
        e_dst = dst_s[m]
        e_ef = ef_s[m]

        ef5 = np.concatenate(
            [np.log1p(e_ef).astype(np.float64),
             np.ones((e_ef.shape[0], 1), dtype=np.float64)], axis=1)
        corr = (ef5 @ M5.T).astype(np.float32)        # [edges, 128]

        xsdt_f = np.zeros((E_pad, 2 * D_NODE), dtype=np.float32)
        xsdt_f[slot, :D_NODE] = x[e_src]
        xsdt_f[slot, D_NODE:] = x[e_dst]
        xsdt_f[slot] += corr
        xsdt_h = np.ascontiguousarray(xsdt_f.T.astype(nbf16))

        amat_h = np.zeros((P, E_pad), dtype=nfp8)
        dstl = ((e_dst % N_CORE) % BLK).astype(np.int64)
        amat_h[slot % P, (slot // P) * P + dstl] = 1.0

        deg_n = np.bincount(e_dst % N_CORE, minlength=N_CORE_PAD).astype(np.float32)
        deg_h = np.zeros((1, N_CORE_PAD), dtype=nbf16)
        xt_h = np.zeros((N_CORE_PAD, D_NODE), dtype=nbf16)
        for s in range(N_BLOCKS):
            b = blk_order[c][s]
            n0 = b * BLK
            n1 = min(n0 + BLK, N_CORE)
            xt_h[s * BLK:s * BLK + (n1 - n0)] = xbf[c * N_CORE + n0:c * N_CORE + n1]
            deg_h[0, s * BLK:s * BLK + (n1 - n0)] = deg_n[n0:n1]
        xtp_h = np.zeros((P, N_CORE_PAD), dtype=nbf16)
        xtp_h[:D_NODE] = xt_h.T
        xtp_h[D_NODE] = deg_h[0]

        in_maps.append({
            "xsdt": xsdt_h, "amat": amat_h, "xtp": xtp_h,
            "w1ab": w1ab_h, "vw": v_h,
            "u1a": u1a_h, "u2": u2_h,
            "bu1c": bu1_h, "bu2c": bu2_h,
        })

    _install_trace_hook_if_possible()
    nc = _build_program(chunk_meta, C, E_pad)
    res = run_bass_kernel_spmd(nc, in_maps, list(range(N_CORES)))
    global _last_results
    _last_results = res

    bu2v = np.asarray(bu2, dtype=np.float32).reshape(1, D_NODE)
    out = np.empty((N_NODES, D_NODE), dtype=np.float32)
    for c in range(N_CORES):
        # bu2 is re-added here (dropped from the device program)
        ot = res.results[c]["outt"].T + bu2v  # [N_CORE_PAD, 64] slot order
        for s in range(N_BLOCKS):
            b = blk_order[c][s]
            n0 = b * BLK
            n1 = min(n0 + BLK, N_CORE)
            out[c * N_CORE + n0:c * N_CORE + n1] = ot[s * BLK:s * BLK + (n1 - n0)]
    return out
